# revision 15
# baseline (speedup 1.0000x reference)
"""AnchorDiffNet fused attention kernel for 8 TRN2 NeuronCores.

Data-parallel over batch: B=8 samples -> 8 cores, no collectives.

Per core (one sample, C=128 channels, M=H*W=4096 positions):
  p0 = softmax(scale * ref^T cur), p1 = softmax(scale * cur^T cur)
  feats0 = cur @ p0^T, feats1 = cur @ p1^T
  y = lrelu(w1' @ [feats0; feats1; cur] + b1')   (BN folded into w1/b1)
  pred = w2 @ y + b2

Dataflow (flash-style, nothing M x M ever touches HBM):
  - S^T chunks [n=128, m=512] = matmul(lhsT=curs[:, nchunk], rhs=mov[:, msuper])
    with curs pre-scaled by SCALE/32, so PSUM holds u = S*SCALE/32. Both
    attentions share the stationary chunk (consecutive matmuls).
  - E^T = exp(32u): no max-subtraction needed (logits ~N(0,1), |max| ~ 6).
    24/32 chunks on ScalarE (Exp, scale=32); 8/32 on VectorE via two custom
    DVE ops: seed = ((u+1)^2+3)(u+1)+2 = 6e^u + O(u^4), then seed^32 * 6^-32.
  - E^T stored fp8e4m3 (exp(S) <= ~365 < 448 max, softmax is scale-consistent).
  - PV in fp8 DoubleRow (K=256/step): out[m,129] = sum_n E^T[n,m]^T @ [curT|1];
    the all-ones column yields the softmax denominator r[m] for free.
  - normalize by 1/r (per-partition), PE-transpose feats^T -> feats, head.
  - Supers are software-pipelined: S/exp for super s overlaps PV/head for s-1.
"""

import sys

sys.path.insert(0, "/opt/trn_rl_repo")

import numpy as np
import ml_dtypes

from concourse import bass, bacc, tile, mybir
from concourse.bass_utils import run_bass_kernel_spmd

B, C, H, W = 8, 128, 64, 64
M = H * W            # 4096
SUP = 512            # m-super width (moving free dim)
NSUP = M // SUP      # 8
CH = 128             # n-chunk width (contraction tile)
NCH = M // CH        # 32
SCALE = float(C) ** -0.5
F32 = mybir.dt.float32
BF16 = mybir.dt.bfloat16
FP8 = mybir.dt.float8e4
BF = ml_dtypes.bfloat16
FP8NP = ml_dtypes.float8_e4m3
FX = mybir.ActivationFunctionType
OP = mybir.AluOpType
DR = mybir.MatmulPerfMode.DoubleRow

# ---------- custom DVE exp ops (registered once at import) ----------
from concourse.dve_spec import Spec, Src0, One, C0, C1, C2, lower, sq
from concourse.dve_uop import DveOpSpec
from concourse.dve_ops import (
    DveOp,
    OPS,
    CUSTOM_DVE_SPECS,
    _SUB_OPCODE_FOR_NAME,
    _CUSTOM_DVE_ROW_BASE,
)


def _register(name, spec):
    if name in _SUB_OPCODE_FOR_NAME:
        return next(op for op in OPS if op.name == name)
    probe = DveOpSpec(name=name, opcode=0, uops=lower(spec, ver="v3"), rd1_en=False)
    op = DveOp(name, spec, subdim=False, uops_sha={"v3": probe.sha("v3")})
    OPS.append(op)
    CUSTOM_DVE_SPECS[name] = spec
    _SUB_OPCODE_FOR_NAME[name] = _CUSTOM_DVE_ROW_BASE + len(OPS) - 1
    return op


# seed = ((u+c2)^2 + 3)(u+c2) + 2 = 6*e^(u+c2-1) + O((u+c2-1)^4); c2 = 1-ln2/32
# folds the E/2 safety-halving into the exponent (softmax is scale-invariant).
_t = Src0 + C2
EXP_SEED = _register(
    "ANT_EXP_SEED",
    Spec(
        body=(sq(_t) + C0) * _t + C1,
        reference=lambda in0, in1, c0, c1, c2: (
            ((in0.astype(np.float64) + c2) ** 2 + c0) * (in0.astype(np.float64) + c2)
            + c1
        ).astype(np.float32),
    ),
)

_s = Src0
for _ in range(5):
    _s = sq(_s)
EXP_FINISH = _register(
    "ANT_EXP_FINISH",
    Spec(
        body=_s * C2,
        reference=lambda in0, in1, c0, c1, c2: (
            in0.astype(np.float64) ** 32 * c2
        ).astype(np.float32),
    ),
)


def build(debug=False):
    nc = bacc.Bacc("TRN2", target_bir_lowering=False, debug=debug, num_devices=8)

    ref_d = nc.dram_tensor("refm", (C, M), BF16, kind="ExternalInput")
    cur_d = nc.dram_tensor("curm", (C, M), BF16, kind="ExternalInput")
    curs_d = nc.dram_tensor("curs", (C, M), BF16, kind="ExternalInput")
    ct8_d = nc.dram_tensor("ct8", (C, NCH * 129), FP8, kind="ExternalInput")
    ctb_d = nc.dram_tensor("ctb", (C, NCH * 129), BF16, kind="ExternalInput")
    w1t_d = nc.dram_tensor("w1t", (C, 3 * C), BF16, kind="ExternalInput")
    b1p_d = nc.dram_tensor("b1p", (C, 1), F32, kind="ExternalInput")
    w2t_d = nc.dram_tensor("w2t", (C, 1), BF16, kind="ExternalInput")
    b2s_d = nc.dram_tensor("b2s", (1, 1), F32, kind="ExternalInput")
    idn_d = nc.dram_tensor("idn", (C, C), BF16, kind="ExternalInput")
    out_d = nc.dram_tensor("out", (1, M), F32, kind="ExternalOutput")

    with tile.TileContext(nc) as tc:
        with (
            tc.tile_pool(name="const", bufs=1) as cp,
            tc.tile_pool(name="et", bufs=2) as etp,
            tc.tile_pool(name="seed", bufs=3) as sdp,
            tc.tile_pool(name="work", bufs=2) as wp,
            tc.tile_pool(name="ftt", bufs=9) as ftp,
            tc.tile_pool(name="psS", bufs=2, space="PSUM") as psS,
            tc.tile_pool(name="ps2", bufs=4, space="PSUM") as ps2,
        ):
            ref_sb = cp.tile([C, M], BF16, tag="ref")
            cur_sb = cp.tile([C, M], BF16, tag="cur")
            curs_sb = cp.tile([C, M], BF16, tag="curs")
            ct8_sb = cp.tile([C, NCH, 129], FP8, tag="ct8")
            ctb_sb = cp.tile([C, NCH, 129], BF16, tag="ctb")
            w1t_sb = cp.tile([C, 3 * C], BF16, tag="w1t")
            b1p_sb = cp.tile([C, 1], F32, tag="b1p")
            w2t_sb = cp.tile([C, 1], BF16, tag="w2t")
            b2s_sb = cp.tile([1, 1], F32, tag="b2s")
            idn_sb = cp.tile([C, C], BF16, tag="idn")
            nl2_sb = cp.tile([C, 1], F32, tag="nl2")
            nc.vector.memset(nl2_sb[:], -float(np.log(2.0)))
            nc.sync.dma_start(ref_sb[:], ref_d.ap())
            nc.sync.dma_start(cur_sb[:], cur_d.ap())
            nc.sync.dma_start(curs_sb[:], curs_d.ap())
            nc.sync.dma_start(ct8_sb[:], ct8_d.ap().rearrange("c (k j) -> c k j", k=NCH))
            nc.sync.dma_start(ctb_sb[:], ctb_d.ap().rearrange("c (k j) -> c k j", k=NCH))
            nc.sync.dma_start(w1t_sb[:], w1t_d.ap())
            nc.sync.dma_start(b1p_sb[:], b1p_d.ap())
            nc.sync.dma_start(w2t_sb[:], w2t_d.ap())
            nc.sync.dma_start(b2s_sb[:], b2s_d.ap())
            nc.sync.dma_start(idn_sb[:], idn_d.ap())

            ets = {}
            LN2 = float(np.log(2.0))

            def emit_s_exp(s):
                """S^T matmuls + exp for super s.

                Attention 0 (cross): logits in ~[-6,6] -> E/2 fits fp8e4m3.
                Attention 1 (self): diagonal logits ~||cur_m||^2/sqrt(C) ~ 11-18
                -> E overflows fp8, store bf16 instead.
                """
                ms = slice(s * SUP, (s + 1) * SUP)
                et0 = etp.tile([C, NCH, SUP], FP8, tag="et0", name=f"et0_{s}")
                et1 = etp.tile([C, NCH, SUP], BF16, tag="et1", name=f"et1_{s}")
                ets[s] = (et0, et1)
                for a, (et, mov) in enumerate(((et0, ref_sb), (et1, cur_sb))):
                    for j in range(NCH // 2):
                        ps = psS.tile([C, 2, SUP], F32, tag="s", name=f"ps{s}_{a}_{j}")
                        for d in range(2):
                            k = 2 * j + d
                            nc.tensor.matmul(
                                ps[:, d, :],
                                lhsT=curs_sb[:, k * CH : (k + 1) * CH],
                                rhs=mov[:, ms],
                                start=True,
                                stop=True,
                            )
                        dst = et[:, 2 * j : 2 * j + 2, :]
                        if j % 4 == 1:
                            sd = sdp.tile([C, 2, SUP], F32, tag="sd", name=f"sd{s}_{a}_{j}")
                            nc.vector._custom_dve(EXP_SEED, out=sd[:], in0=ps[:],
                                                  s0=3.0, s1=2.0, imm2=1.0 - LN2 / 32)
                            nc.vector._custom_dve(EXP_FINISH, out=dst, in0=sd[:],
                                                  imm2=6.0 ** -32)
                        else:
                            nc.scalar.activation(dst, ps[:], FX.Exp,
                                                 scale=32.0, bias=nl2_sb[:])

            def emit_pv_head(s):
                """PV (fp8 DR for a=0, bf16 for a=1) + normalize + transpose + head."""
                ms = slice(s * SUP, (s + 1) * SUP)
                et0, et1 = ets.pop(s)
                ftts = [[None] * 4, [None] * 4]
                for a in range(2):
                    pvs = [
                        ps2.tile([C, 129], F32, tag="acc", name=f"pv{s}_{a}_{i}")
                        for i in range(4)
                    ]
                    if a == 0:
                        for j in range(NCH // 2):
                            for mb in range(4):
                                nc.tensor.matmul(
                                    pvs[mb][:],
                                    lhsT=et0[:, 2 * j : 2 * j + 2, mb * CH : (mb + 1) * CH],
                                    rhs=ct8_sb[:, 2 * j : 2 * j + 2, :],
                                    start=(j == 0),
                                    stop=(j == NCH // 2 - 1),
                                    perf_mode=DR,
                                )
                    else:
                        for k in range(NCH):
                            for mb in range(4):
                                nc.tensor.matmul(
                                    pvs[mb][:],
                                    lhsT=et1[:, k, mb * CH : (mb + 1) * CH],
                                    rhs=ctb_sb[:, k, :],
                                    start=(k == 0),
                                    stop=(k == NCH - 1),
                                )
                    for mb in range(4):
                        rr = ftp.tile([C, 1], F32, tag="rr", name=f"rr{s}_{a}_{mb}")
                        nc.vector.reciprocal_approx_fast(rr[:], pvs[mb][:, 128:129])
                        ftt = ftp.tile([C, C], BF16, tag=f"ftt{a}", name=f"ftt{s}_{a}_{mb}")
                        nc.vector.tensor_scalar(
                            ftt[:], pvs[mb][:, 0:128], rr[:], None, OP.mult
                        )
                        ftts[a][mb] = ftt
                feats = [None, None]
                for a in range(2):
                    f = wp.tile([C, SUP], BF16, tag=f"feats{a}", name=f"feats{s}_{a}")
                    for mb in range(4):
                        pt = ps2.tile([C, C], BF16, tag="acc", name=f"pt{s}_{a}_{mb}")
                        nc.tensor.transpose(pt[:], ftts[a][mb][:], idn_sb[:])
                        nc.vector.tensor_copy(f[:, mb * CH : (mb + 1) * CH], pt[:])
                    feats[a] = f
                py = ps2.tile([C, SUP], F32, tag="acc", name=f"py{s}")
                for kc in range(3):
                    rhs = cur_sb[:, ms] if kc == 2 else feats[kc][:]
                    nc.tensor.matmul(
                        py[:],
                        lhsT=w1t_sb[:, kc * C : (kc + 1) * C],
                        rhs=rhs,
                        start=(kc == 0),
                        stop=(kc == 2),
                    )
                yb = wp.tile([C, SUP], F32, tag="yb", name=f"yb{s}")
                nc.vector.tensor_scalar(yb[:], py[:], b1p_sb[:], None, OP.add)
                yt = wp.tile([C, SUP], F32, tag="yt", name=f"yt{s}")
                nc.vector.tensor_scalar(yt[:], yb[:], 0.01, None, OP.mult)
                ym = wp.tile([C, SUP], BF16, tag="ym", name=f"ym{s}")
                nc.vector.tensor_tensor(ym[:], yb[:], yt[:], OP.max)
                pw = ps2.tile([1, SUP], F32, tag="acc", name=f"pw{s}")
                nc.tensor.matmul(pw[:], lhsT=w2t_sb[:], rhs=ym[:], start=True, stop=True)
                ob = wp.tile([1, SUP], F32, tag="ob", name=f"ob{s}")
                nc.vector.tensor_scalar(ob[:], pw[:], b2s_sb[:], None, OP.add)
                nc.sync.dma_start(out_d.ap()[:, ms], ob[:])

            for s in range(NSUP + 1):
                if s < NSUP:
                    emit_s_exp(s)
                if s > 0:
                    emit_pv_head(s - 1)

    nc.compile()
    return nc


def prep_inputs(ref_feat, curr_feat, w1, b1, gamma, beta, running_mean, running_var, w2, b2):
    """Host-side prep: BN fold, scale fold, transposes, casts."""
    ref_feat = np.asarray(ref_feat, np.float32)
    curr_feat = np.asarray(curr_feat, np.float32)
    w1 = np.asarray(w1, np.float32)
    inv = np.asarray(gamma, np.float32) / np.sqrt(np.asarray(running_var, np.float32) + 1e-5)
    w1p = w1 * inv[:, None]
    b1p = (np.asarray(b1, np.float32) * inv + np.asarray(beta, np.float32)
           - np.asarray(running_mean, np.float32) * inv)
    w1t = np.ascontiguousarray(
        w1p.reshape(C, 3, C).transpose(2, 1, 0).reshape(C, 3 * C)
    ).astype(BF)
    w2t = np.ascontiguousarray(np.asarray(w2, np.float32).T).astype(BF)
    b2s = np.asarray(b2, np.float32).reshape(1, 1)
    b1p = b1p.reshape(C, 1)
    idn = np.eye(C, dtype=np.float32).astype(BF)
    ssc = np.float32(SCALE / 32.0)

    in_maps = []
    for b in range(B):
        ref_m = ref_feat[b].reshape(C, M)
        cur_m = curr_feat[b].reshape(C, M)
        t = cur_m.reshape(C, NCH, CH).transpose(2, 1, 0)  # (n_in_chunk, k, c)
        ct1 = np.concatenate([t, np.ones((CH, NCH, 1), np.float32)], axis=2)
        in_maps.append({
            "refm": ref_m.astype(BF),
            "curm": cur_m.astype(BF),
            "curs": (cur_m * ssc).astype(BF),
            "ct8": np.ascontiguousarray(ct1.reshape(CH, NCH * 129)).astype(FP8NP),
            "ctb": np.ascontiguousarray(ct1.reshape(CH, NCH * 129)).astype(BF),
            "w1t": w1t,
            "b1p": b1p,
            "w2t": w2t,
            "b2s": b2s,
            "idn": idn,
        })
    return in_maps


_NC = None


def kernel(**inputs):
    global _NC
    if _NC is None:
        _NC = build(debug=False)
    in_maps = prep_inputs(**inputs)
    res = run_bass_kernel_spmd(_NC, in_maps, core_ids=list(range(B)))
    out = np.stack([np.asarray(res.results[i]["out"], np.float32).reshape(1, H, W)
                    for i in range(B)])
    return out
